# revision 1
# baseline (speedup 1.0000x reference)
"""Causal self-attention (GQA, rope, qk-rmsnorm) Trainium2 kernel, 8 NeuronCores.

Sharding: core = (b, g), b = core // 4 (batch), g = core % 4.
Each core handles query row-chunks {g, 4+g, 8+g, 12+g} (128 rows each) of its
batch: computes Q for those 512 rows, K/V for all keys it needs (duplicated),
attention for all 16 heads, and its 512-row slice of the output projection.
Host gathers row slices. The program is identical on all cores (SPMD); all
per-core variation comes through the input shards.

Slot c (c = 0..3) covers query chunk 4c+g with keys [0, 512*(c+1)) — uniform
across cores; causal masking inside the last 512 keys comes from a
host-provided additive mask shard.
"""

import sys

if "/opt/trn_rl_repo" not in sys.path:
    sys.path.insert(0, "/opt/trn_rl_repo")

import numpy as np

B, T, C = 2, 2048, 2048
NH, NKV = 16, 4
HD = C // NH  # 128
P = 128
NT = T // P            # 16 token tiles per batch
NCT = C // P           # 16 contraction tiles
QROWS = 512            # own query rows per core
NQT = QROWS // P       # 4 own token tiles
KLEN = [512, 1024, 1536, 2048]   # keys per slot
SCALE = 1.0 / float(np.sqrt(HD))
EPS = float(np.finfo(np.float32).eps)
NEG = -1.0e9

_CACHE = {}


def _chunks(g):
    return [g, 4 + g, 8 + g, 12 + g]


def _rows(g):
    return np.concatenate([np.arange(ch * P, (ch + 1) * P) for ch in _chunks(g)])


def _qmask_t(g):
    """Additive mask, transposed layout: (slot c, sub s, k_in_sub i, q j).

    For slot c the score tile is S^T[k, q] with k in [0, KLEN[c]) and q the
    128 rows of chunk 4c+g. Only keys in the last 512 of the slot can be
    invalid; mask[c, s, i, j] = 0 if key (KLEN[c]-512 + s*128 + i) <= query
    (128*(4c+g) + j) else NEG.
    """
    m = np.zeros((4, 4, P, P), np.float32)
    for c in range(4):
        k0 = KLEN[c] - 512
        r0 = (4 * c + g) * P
        k = k0 + np.arange(512)[:, None]          # (512, 1)
        q = r0 + np.arange(P)[None, :]            # (1, 128)
        m[c] = np.where(k <= q, 0.0, NEG).reshape(4, P, P)
    return m


def _build():
    import concourse.bacc as bacc
    import concourse.bass as bass
    import concourse.mybir as mybir
    import concourse.tile as tile
    from concourse.masks import make_identity

    f32 = mybir.dt.float32
    bf16 = mybir.dt.bfloat16
    AF = mybir.ActivationFunctionType
    OP = mybir.AluOpType
    AX = mybir.AxisListType

    nc = bacc.Bacc("TRN2", target_bir_lowering=False, debug=False, num_devices=8)

    xf = nc.dram_tensor("xf", [T, C], f32, kind="ExternalInput").ap()
    xo = nc.dram_tensor("xo", [QROWS, C], f32, kind="ExternalInput").ap()
    cosf = nc.dram_tensor("cosf", [T, HD // 2], f32, kind="ExternalInput").ap()
    sinf = nc.dram_tensor("sinf", [T, HD // 2], f32, kind="ExternalInput").ap()
    coso = nc.dram_tensor("coso", [QROWS, HD // 2], f32, kind="ExternalInput").ap()
    sino = nc.dram_tensor("sino", [QROWS, HD // 2], f32, kind="ExternalInput").ap()
    wq = nc.dram_tensor("wq", [C, C], f32, kind="ExternalInput").ap()
    wk = nc.dram_tensor("wk", [C, NKV * HD], f32, kind="ExternalInput").ap()
    wv = nc.dram_tensor("wv", [C, NKV * HD], f32, kind="ExternalInput").ap()
    wo = nc.dram_tensor("wo", [C, C], f32, kind="ExternalInput").ap()
    qm = nc.dram_tensor("qm", [4, 4, P, P], f32, kind="ExternalInput").ap()
    yo = nc.dram_tensor("yo", [QROWS, C], f32, kind="ExternalOutput").ap()

    def bcast4(ap2d):
        # [128, 64] -> [128, 4, 64] with middle step 0 (replicate across heads)
        return bass.AP(
            tensor=ap2d.tensor,
            offset=ap2d.offset,
            ap=[ap2d.ap[0], [0, 4], ap2d.ap[1]],
        )

    with tile.TileContext(nc) as tc:
        with (
            tc.tile_pool(name="singles", bufs=1) as singles,
            tc.tile_pool(name="big", bufs=1) as bigpool,
            tc.tile_pool(name="xb", bufs=2) as xbpool,
            tc.tile_pool(name="wsl", bufs=2) as wslpool,
            tc.tile_pool(name="cs", bufs=2) as cspool,
            tc.tile_pool(name="epi", bufs=2) as epipool,
            tc.tile_pool(name="qh", bufs=3) as qhpool,
            tc.tile_pool(name="wo3", bufs=2) as wopool,
            tc.tile_pool(name="pt", bufs=4) as ptpool,
            tc.tile_pool(name="smallf", bufs=2) as smallf,
            tc.tile_pool(name="outs", bufs=2) as outpool,
            tc.tile_pool(name="psS", bufs=3, space="PSUM") as psS,
            tc.tile_pool(name="psY", bufs=3, space="PSUM") as psY,
            tc.tile_pool(name="psD", bufs=2, space="PSUM") as psD,
        ):
            ident = singles.tile([P, P], bf16)
            make_identity(nc, ident)
            identf = singles.tile([P, P], f32)
            make_identity(nc, identf)
            ones128 = singles.tile([P, P], bf16)
            nc.vector.memset(ones128, 1.0)
            eps_q = singles.tile([P, 1], f32)
            nc.vector.memset(eps_q, EPS)
            eps_k = singles.tile([P, 1], f32)
            nc.vector.memset(eps_k, HD * EPS)

            # persistent big SBUF tensors
            qT = bigpool.tile([P, NH, QROWS], bf16, tag="qT")      # [d, h, q]
            kT = bigpool.tile([P, NKV, T], bf16, tag="kT")         # [d, kvh, k]
            vA = bigpool.tile([P, NT, NKV, HD], bf16, tag="vA")    # [ktok, tt, kvh, d]
            yT = bigpool.tile([P, NCT, QROWS], bf16, tag="yT")     # [d, ct, q]
            qmask = singles.tile([P, 4, 4, P], bf16)               # [ki, c, sub, q]
            nc.gpsimd.dma_start(out=qmask, in_=qm.rearrange("c s i j -> i c s j"))

            # ---------------- helpers ----------------
            def x_transpose(src_ap, dst4, dst_col, name):
                """Load one 128-row strip of x (f32 dram) via HWDGE, transpose
                (f32) into the 4 [P, 4, width] bf16 dst tiles at dst_col."""
                for h2 in range(2):
                    xb = xbpool.tile([P, 8 * P], f32, tag="xb", name=f"xb{name}{h2}")
                    nc.sync.dma_start(
                        out=xb, in_=src_ap[:, h2 * 1024:(h2 + 1) * 1024]
                    )
                    for gr in (2 * h2, 2 * h2 + 1):
                        ptr = psY.tile([P, 512], f32, tag="Y", name=f"ptr{name}{gr}")
                        for j in range(4):
                            ctl = 4 * gr + j - 8 * h2
                            nc.tensor.transpose(
                                ptr[:, j * P:(j + 1) * P],
                                xb[:, ctl * P:(ctl + 1) * P],
                                identf,
                            )
                        nc.scalar.copy(
                            dst4[gr][:, :, dst_col:dst_col + P],
                            ptr.rearrange("p (s n) -> p s n", s=4),
                        )

            def load_w_slab(w_ap, col0, name):
                """One 512-col slab of a weight, as [128, 16, 512] bf16."""
                wsl = wslpool.tile([P, NCT, 512], bf16, tag="wsl", name=name)
                for gr in range(4):
                    nc.gpsimd.dma_start(
                        out=wsl[:, 4 * gr:4 * gr + 4, :],
                        in_=w_ap[:, col0:col0 + 512].rearrange(
                            "(a p) n -> p a n", p=P
                        )[:, 4 * gr:4 * gr + 4, :],
                    )
                return wsl

            pending = []  # delayed PE transpose packs (2-deep pipeline)

            def drain_pending(keep=0):
                while len(pending) > keep:
                    pending.pop(0)()

            def rope_rms(ps, cos4, sin4, out_bf, eps_ap, sqrt_scale):
                """ps: [128, 512] psum f32 (4 heads). Writes normalized bf16
                rope output to out_bf [128, 4, 128]."""
                v3 = ps.rearrange("p (h d) -> p h d", h=4)
                ro = epipool.tile([P, 4, HD], f32, tag="ro", name="ro")
                cs = epipool.tile([P, 4, HD], f32, tag="cs", name="cs")
                sn = epipool.tile([P, 4, HD], f32, tag="sn", name="sn")
                nc.vector.tensor_tensor(cs[:, :, 0:64], v3[:, :, 0:64], cos4, op=OP.mult)
                nc.vector.tensor_tensor(cs[:, :, 64:128], v3[:, :, 64:128], cos4, op=OP.mult)
                nc.vector.tensor_tensor(sn[:, :, 0:64], v3[:, :, 0:64], sin4, op=OP.mult)
                nc.vector.tensor_tensor(sn[:, :, 64:128], v3[:, :, 64:128], sin4, op=OP.mult)
                nc.vector.tensor_tensor(ro[:, :, 0:64], cs[:, :, 0:64], sn[:, :, 64:128], op=OP.add)
                nc.vector.tensor_sub(ro[:, :, 64:128], cs[:, :, 64:128], sn[:, :, 0:64])
                ss = smallf.tile([P, 4], f32, tag="ss", name="ss")
                sq = epipool.tile([P, 4, HD], f32, tag="cs", name="sq")
                nc.vector.tensor_tensor(sq, ro, ro, op=OP.mult)
                nc.vector.reduce_sum(ss, sq, axis=AX.X)
                rms = smallf.tile([P, 4], f32, tag="rms", name="rms")
                nc.scalar.activation(rms, ss, AF.Sqrt, bias=eps_ap, scale=sqrt_scale)
                rinv = smallf.tile([P, 4], f32, tag="rms", name="rinv")
                nc.vector.reciprocal_approx_fast(rinv, rms)
                for hh in range(4):
                    nc.vector.tensor_scalar_mul(
                        out_bf[:, hh, :], ro[:, hh, :], rinv[:, hh:hh + 1]
                    )

            def pack_transpose(src_bf, dst):
                """src_bf [128, 4, 128] bf16 -> 4 PE transposes -> one copy to
                dst ([128, 4, 128] view)."""
                ptr = psY.tile([P, 512], bf16, tag="Y", name="ptrq")
                for hh in range(4):
                    nc.tensor.transpose(
                        ptr[:, hh * P:(hh + 1) * P], src_bf[:, hh, :], ident
                    )
                nc.vector.tensor_copy(dst, ptr.rearrange("p (s n) -> p s n", s=4))

            def cos_tiles(cap, sap, t0, name):
                cos4 = cspool.tile([P, 4, 64], f32, tag="cs4", name=f"c{name}")
                sin4 = cspool.tile([P, 4, 64], f32, tag="sn4", name=f"s{name}")
                nc.gpsimd.dma_start(out=cos4, in_=bcast4(cap[t0:t0 + P, :]))
                nc.gpsimd.dma_start(out=sin4, in_=bcast4(sap[t0:t0 + P, :]))
                return cos4, sin4

            # ---------------- phase 0a + 1Q: own-row transposes, Q proj ----
            xoT = [
                bigpool.tile([P, 4, QROWS], bf16, tag=f"xT{gr}", name=f"xoT{gr}")
                for gr in range(4)
            ]
            for tt in range(NQT):
                x_transpose(xo[tt * P:(tt + 1) * P, :], xoT, tt * P, f"o{tt}")

            for s in range(4):
                wsl = load_w_slab(wq, s * 512, f"wq{s}")
                for tt in range(NQT):
                    ps = psS.tile([P, 512], f32, tag="S", name="psq")
                    for kt in range(NCT):
                        nc.tensor.matmul(
                            ps,
                            xoT[kt // 4][:, kt % 4, tt * P:(tt + 1) * P],
                            wsl[:, kt, :],
                            start=(kt == 0),
                            stop=(kt == NCT - 1),
                        )
                    cos4, sin4 = cos_tiles(coso, sino, tt * P, f"q{s}{tt}")
                    qhat = qhpool.tile([P, 4, HD], bf16, tag="qhat", name="qhat")
                    rope_rms(ps, cos4, sin4, qhat, eps_q, 1.0 / HD)
                    drain_pending(1)
                    pending.append(
                        lambda qhat=qhat, s=s, tt=tt: pack_transpose(
                            qhat,
                            qT[:, 4 * s:4 * s + 4, (3 - tt) * P:(4 - tt) * P],
                        )
                    )
            drain_pending()

            # ---------------- phase 0b + 1KV: two halves ----------------
            for half in range(2):
                xfT = [
                    bigpool.tile([P, 4, 8 * P], bf16, tag=f"xT{gr}",
                                 name=f"xfT{half}{gr}")
                    for gr in range(4)
                ]
                for tt in range(8 * half, 8 * half + 8):
                    x_transpose(xf[tt * P:(tt + 1) * P, :], xfT,
                                (tt - 8 * half) * P, f"f{tt}")
                wslk = load_w_slab(wk, 0, f"wk{half}")
                for tt in range(8 * half, 8 * half + 8):
                    tl = tt - 8 * half
                    ps = psS.tile([P, 512], f32, tag="S", name="psk")
                    for kt in range(NCT):
                        nc.tensor.matmul(
                            ps,
                            xfT[kt // 4][:, kt % 4, tl * P:(tl + 1) * P],
                            wslk[:, kt, :],
                            start=(kt == 0),
                            stop=(kt == NCT - 1),
                        )
                    cos4, sin4 = cos_tiles(cosf, sinf, tt * P, f"k{tt}")
                    khat = qhpool.tile([P, 4, HD], bf16, tag="qhat", name="khat")
                    # fold attn scale into k's rms: 1/sqrt(ss + 128*eps)
                    rope_rms(ps, cos4, sin4, khat, eps_k, 1.0)
                    drain_pending(1)
                    pending.append(
                        lambda khat=khat, tt=tt: pack_transpose(
                            khat, kT[:, 0:4, tt * P:(tt + 1) * P]
                        )
                    )
                drain_pending()
                wslv = load_w_slab(wv, 0, f"wv{half}")
                for tt in range(8 * half, 8 * half + 8):
                    tl = tt - 8 * half
                    psv = psS.tile([P, 512], f32, tag="S", name="psv")
                    for kt in range(NCT):
                        nc.tensor.matmul(
                            psv,
                            xfT[kt // 4][:, kt % 4, tl * P:(tl + 1) * P],
                            wslv[:, kt, :],
                            start=(kt == 0),
                            stop=(kt == NCT - 1),
                        )
                    nc.scalar.copy(
                        vA[:, tt, :, :], psv.rearrange("p (h d) -> p h d", h=4)
                    )

            # ---------------- phase 2: attention (scores-transposed) -------
            # Head-major: for each head, one variable-width matmul per
            # key-tile streams all still-valid query slots at once
            # (kt 0-3 -> N=512 ... kt 12-15 -> N=128). yt/den accumulate all
            # four slots per head in single psum tiles.
            tail_state = []  # (yt_psum, den_psum, h)

            def emit_tail():
                if not tail_state:
                    return
                yt, den, h = tail_state.pop(0)
                rinv = smallf.tile([P, QROWS], f32, tag="rq", name="rqinv")
                nc.vector.reciprocal_approx_fast(rinv, den)
                nc.vector.tensor_tensor(yT[:, h, :], yt, rinv, op=OP.mult)

            def emit_dpv(ent, yt, den, kvh):
                ppt, pkt, pn = ent
                last = pkt == NT - 1
                nc.tensor.matmul(
                    den[:, 0:pn], ones128, ppt[:, 0:pn],
                    start=(pkt == 0), stop=last, skip_group_check=True,
                )
                nc.tensor.matmul(
                    yt[:, 0:pn], vA[:, pkt, kvh, :], ppt[:, 0:pn],
                    start=(pkt == 0), stop=last, skip_group_check=True,
                )

            def load_wo_slab(s3):
                w3 = wopool.tile([P, NCT, 512], bf16, tag="wo3", name=f"wo{s3}")
                for gr in range(4):
                    nc.gpsimd.dma_start(
                        out=w3[:, 4 * gr:4 * gr + 4, :],
                        in_=wo[:, s3 * 512:s3 * 512 + 512].rearrange(
                            "(a p) n -> p a n", p=P
                        )[:, 4 * gr:4 * gr + 4, :],
                    )
                return w3

            # prefetch the first two wo slabs; their DMAs run under phase 2
            w3s = {0: load_wo_slab(0), 1: load_wo_slab(1)}

            for h in range(NH):
                kvh = h // (NH // NKV)
                yt = psY.tile([P, QROWS], f32, tag="Y", name="yt")
                den = psD.tile([P, QROWS], f32, tag="D", name="den")
                dq = []  # exp'd tiles awaiting den/PV (2-deep pipeline)
                for kt in range(NT):
                    # q-slot columns are stored high-slot-first, so the
                    # still-valid slots for key tile kt are columns [0, n)
                    n = QROWS - (kt // 4) * P
                    S = psS.tile([P, 512], f32, tag="S", name="Sb")
                    nc.tensor.matmul(
                        S[:, 0:n],
                        kT[:, kvh, kt * P:(kt + 1) * P],
                        qT[:, h, 0:n],
                        start=True,
                        stop=False,
                        skip_group_check=True,
                    )
                    # causal mask for the diagonal slot (last 128 valid cols),
                    # accumulated on the PE via an identity matmul
                    nc.tensor.matmul(
                        S[:, n - P:n], ident, qmask[:, kt // 4, kt % 4, :],
                        start=False, stop=True, skip_group_check=True,
                    )
                    if kt == 0 and tail_state:
                        emit_tail()
                    # attn scale already folded into k's rms normalization
                    pt = ptpool.tile([P, 512], bf16, tag="pt", name="pt")
                    nc.scalar.activation(pt[:, 0:n], S[:, 0:n], AF.Exp, scale=1.0)
                    dq.append((pt, kt, n))
                    if len(dq) > 2:
                        emit_dpv(dq.pop(0), yt, den, kvh)
                while dq:
                    emit_dpv(dq.pop(0), yt, den, kvh)
                tail_state.append((yt, den, h))
            emit_tail()

            # ---------------- phase 3: output projection ----------------
            for s3 in range(4):
                w3 = w3s.pop(s3)
                if s3 + 2 < 4:
                    w3s[s3 + 2] = load_wo_slab(s3 + 2)
                for qt in range(4):
                    ps = psS.tile([P, 512], f32, tag="S", name="ps3")
                    for ct in range(NCT):
                        nc.tensor.matmul(
                            ps,
                            yT[:, ct, (3 - qt) * P:(4 - qt) * P],
                            w3[:, ct, :],
                            start=(ct == 0),
                            stop=(ct == NCT - 1),
                        )
                    ot = outpool.tile([P, 512], f32, tag="ot", name="ot")
                    nc.vector.tensor_copy(ot, ps)
                    nc.sync.dma_start(
                        out=yo[qt * P:(qt + 1) * P, s3 * 512:(s3 + 1) * 512],
                        in_=ot,
                    )

    nc.compile()
    return nc


def _get_nc():
    if "nc" not in _CACHE:
        _CACHE["nc"] = _build()
    return _CACHE["nc"]


def _in_maps(x, cosr, sinr, wq, wk, wv, wo):
    maps = []
    for core in range(8):
        b, g = core // 4, core % 4
        rows = _rows(g)
        maps.append({
            "xf": np.ascontiguousarray(x[b]),
            "xo": np.ascontiguousarray(x[b][rows]),
            "cosf": cosr,
            "sinf": sinr,
            "coso": np.ascontiguousarray(cosr[rows]),
            "sino": np.ascontiguousarray(sinr[rows]),
            "wq": wq, "wk": wk, "wv": wv, "wo": wo,
            "qm": _qmask_t(g),
        })
    return maps


def kernel(x, cos, sin, wq, wk, wv, wo):
    from concourse.bass_utils import run_bass_kernel_spmd

    x = np.ascontiguousarray(np.asarray(x, np.float32))
    cosr = np.ascontiguousarray(np.asarray(cos, np.float32).reshape(T, HD // 2))
    sinr = np.ascontiguousarray(np.asarray(sin, np.float32).reshape(T, HD // 2))
    wq = np.ascontiguousarray(np.asarray(wq, np.float32))
    wk = np.ascontiguousarray(np.asarray(wk, np.float32))
    wv = np.ascontiguousarray(np.asarray(wv, np.float32))
    wo = np.ascontiguousarray(np.asarray(wo, np.float32))

    nc = _get_nc()
    maps = _in_maps(x, cosr, sinr, wq, wk, wv, wo)
    _CACHE["in_maps"] = maps
    res = run_bass_kernel_spmd(nc, maps, list(range(8)))
    y = np.empty((B, T, C), np.float32)
    for core in range(8):
        b, g = core // 4, core % 4
        y[b][_rows(g)] = res.results[core]["yo"]
    return y



# revision 3
# speedup vs baseline: 1.1825x; 1.1825x over previous
"""Causal self-attention (GQA, rope, qk-rmsnorm) Trainium2 kernel, 8 NeuronCores.

Sharding: core = (b, g), b = core // 4 (batch), g = core % 4.
Each core handles query row-chunks {g, 4+g, 8+g, 12+g} (128 rows each) of its
batch: computes Q for those 512 rows, K/V for ONLY its contiguous 512-key
shard [512g, 512(g+1)), then AllGathers the rope+rms'd K-hat and raw V
across the 4 cores of its batch (replica groups [[0..3],[4..7]]) so every
core has the full 2048 keys.  Attention for all 16 heads and the 512-row
slice of the output projection follow as before.  Host gathers row slices.
The program is identical on all cores (SPMD); all per-core variation comes
through the input shards.

Slot c (c = 0..3) covers query chunk 4c+g with keys [0, 512*(c+1)) — uniform
across cores; causal masking inside the last 512 keys comes from a
host-provided additive mask shard.
"""

import sys

if "/opt/trn_rl_repo" not in sys.path:
    sys.path.insert(0, "/opt/trn_rl_repo")

import numpy as np

B, T, C = 2, 2048, 2048
NH, NKV = 16, 4
HD = C // NH  # 128
P = 128
NT = T // P            # 16 token tiles per batch
NCT = C // P           # 16 contraction tiles
QROWS = 512            # own query rows per core
NQT = QROWS // P       # 4 own token tiles
KLEN = [512, 1024, 1536, 2048]   # keys per slot
SCALE = 1.0 / float(np.sqrt(HD))
EPS = float(np.finfo(np.float32).eps)
NEG = -1.0e9

_CACHE = {}


def _chunks(g):
    return [g, 4 + g, 8 + g, 12 + g]


def _rows(g):
    return np.concatenate([np.arange(ch * P, (ch + 1) * P) for ch in _chunks(g)])


def _qmask_t(g):
    """Additive mask, transposed layout: (slot c, sub s, k_in_sub i, q j).

    For slot c the score tile is S^T[k, q] with k in [0, KLEN[c]) and q the
    128 rows of chunk 4c+g. Only keys in the last 512 of the slot can be
    invalid; mask[c, s, i, j] = 0 if key (KLEN[c]-512 + s*128 + i) <= query
    (128*(4c+g) + j) else NEG.
    """
    m = np.zeros((4, 4, P, P), np.float32)
    for c in range(4):
        k0 = KLEN[c] - 512
        r0 = (4 * c + g) * P
        k = k0 + np.arange(512)[:, None]          # (512, 1)
        q = r0 + np.arange(P)[None, :]            # (1, 128)
        m[c] = np.where(k <= q, 0.0, NEG).reshape(4, P, P)
    return m


def _build():
    import concourse.bacc as bacc
    import concourse.bass as bass
    import concourse.mybir as mybir
    import concourse.tile as tile
    from concourse.masks import make_identity

    f32 = mybir.dt.float32
    bf16 = mybir.dt.bfloat16
    AF = mybir.ActivationFunctionType
    OP = mybir.AluOpType
    AX = mybir.AxisListType

    nc = bacc.Bacc("TRN2", target_bir_lowering=False, debug=False, num_devices=8)

    xo = nc.dram_tensor("xo", [QROWS, C], f32, kind="ExternalInput").ap()
    xk = nc.dram_tensor("xk", [QROWS, C], f32, kind="ExternalInput").ap()
    coso = nc.dram_tensor("coso", [QROWS, HD // 2], f32, kind="ExternalInput").ap()
    sino = nc.dram_tensor("sino", [QROWS, HD // 2], f32, kind="ExternalInput").ap()
    cosk = nc.dram_tensor("cosk", [QROWS, HD // 2], f32, kind="ExternalInput").ap()
    sink = nc.dram_tensor("sink", [QROWS, HD // 2], f32, kind="ExternalInput").ap()
    wq = nc.dram_tensor("wq", [C, C], f32, kind="ExternalInput").ap()
    wk = nc.dram_tensor("wk", [C, NKV * HD], f32, kind="ExternalInput").ap()
    wv = nc.dram_tensor("wv", [C, NKV * HD], f32, kind="ExternalInput").ap()
    wo = nc.dram_tensor("wo", [C, C], f32, kind="ExternalInput").ap()
    qm = nc.dram_tensor("qm", [4, 4, P, P], f32, kind="ExternalInput").ap()
    yo = nc.dram_tensor("yo", [QROWS, C], f32, kind="ExternalOutput").ap()

    def bcast4(ap2d):
        # [128, 64] -> [128, 4, 64] with middle step 0 (replicate across heads)
        return bass.AP(
            tensor=ap2d.tensor,
            offset=ap2d.offset,
            ap=[ap2d.ap[0], [0, 4], ap2d.ap[1]],
        )

    with tile.TileContext(nc) as tc:
        with (
            tc.tile_pool(name="singles", bufs=1) as singles,
            tc.tile_pool(name="big", bufs=1) as bigpool,
            tc.tile_pool(name="xb", bufs=2) as xbpool,
            tc.tile_pool(name="wsl", bufs=2) as wslpool,
            tc.tile_pool(name="cs", bufs=2) as cspool,
            tc.tile_pool(name="epi", bufs=2) as epipool,
            tc.tile_pool(name="qh", bufs=3) as qhpool,
            tc.tile_pool(name="wo3", bufs=2) as wopool,
            tc.tile_pool(name="pt", bufs=4) as ptpool,
            tc.tile_pool(name="smallf", bufs=2) as smallf,
            tc.tile_pool(name="outs", bufs=2) as outpool,
            tc.tile_pool(name="dramb", bufs=1, space="DRAM") as drampool,
            tc.tile_pool(name="psS", bufs=3, space="PSUM") as psS,
            tc.tile_pool(name="psY", bufs=3, space="PSUM") as psY,
            tc.tile_pool(name="psD", bufs=2, space="PSUM") as psD,
        ):
            ident = singles.tile([P, P], bf16)
            make_identity(nc, ident)
            identf = singles.tile([P, P], f32)
            make_identity(nc, identf)
            ones128 = singles.tile([P, P], bf16)
            nc.vector.memset(ones128, 1.0)
            eps_q = singles.tile([P, 1], f32)
            nc.vector.memset(eps_q, EPS)
            eps_k = singles.tile([P, 1], f32)
            nc.vector.memset(eps_k, HD * EPS)

            # persistent big SBUF tensors
            qT = bigpool.tile([P, NH, QROWS], bf16, tag="qT")      # [d, h, q]
            kT = bigpool.tile([P, NKV, T], bf16, tag="kT")         # [d, kvh, k]
            vA = bigpool.tile([P, NT, NKV, HD], bf16, tag="vA")    # [ktok, tt, kvh, d]
            kOwn = bigpool.tile([P, NKV, QROWS], bf16, tag="kOwn")  # own shard
            vOwn = bigpool.tile([P, NQT, NKV, HD], bf16, tag="vOwn")
            yT = bigpool.tile([P, NCT, QROWS], bf16, tag="yT")     # [d, ct, q]
            qmask = singles.tile([P, 4, 4, P], bf16)               # [ki, c, sub, q]
            nc.gpsimd.dma_start(out=qmask, in_=qm.rearrange("c s i j -> i c s j"))

            # DRAM bounce buffers for the K/V AllGather (bf16)
            kv_in = drampool.tile([P, 4096], bf16)                 # 1 MB
            kv_out = drampool.tile([4, P, 4096], bf16)             # 4 MB

            # ---------------- helpers ----------------
            def x_transpose(src_ap, dst4, dst_col, name):
                """Load one 128-row strip of x (f32 dram) via HWDGE, transpose
                (f32) into the 4 [P, 4, width] bf16 dst tiles at dst_col."""
                for h2 in range(2):
                    xb = xbpool.tile([P, 8 * P], f32, tag="xb", name=f"xb{name}{h2}")
                    nc.sync.dma_start(
                        out=xb, in_=src_ap[:, h2 * 1024:(h2 + 1) * 1024]
                    )
                    for gr in (2 * h2, 2 * h2 + 1):
                        ptr = psY.tile([P, 512], f32, tag="Y", name=f"ptr{name}{gr}")
                        for j in range(4):
                            ctl = 4 * gr + j - 8 * h2
                            nc.tensor.transpose(
                                ptr[:, j * P:(j + 1) * P],
                                xb[:, ctl * P:(ctl + 1) * P],
                                identf,
                            )
                        nc.scalar.copy(
                            dst4[gr][:, :, dst_col:dst_col + P],
                            ptr.rearrange("p (s n) -> p s n", s=4),
                        )

            def load_w_slab(w_ap, col0, name):
                """One 512-col slab of a weight, as [128, 16, 512] bf16."""
                wsl = wslpool.tile([P, NCT, 512], bf16, tag="wsl", name=name)
                for gr in range(4):
                    nc.gpsimd.dma_start(
                        out=wsl[:, 4 * gr:4 * gr + 4, :],
                        in_=w_ap[:, col0:col0 + 512].rearrange(
                            "(a p) n -> p a n", p=P
                        )[:, 4 * gr:4 * gr + 4, :],
                    )
                return wsl

            pending = []  # delayed PE transpose packs (2-deep pipeline)

            def drain_pending(keep=0):
                while len(pending) > keep:
                    pending.pop(0)()

            def rope_rms(ps, cos4, sin4, out_bf, eps_ap, sqrt_scale):
                """ps: [128, 512] psum f32 (4 heads). Writes normalized bf16
                rope output to out_bf [128, 4, 128]."""
                v3 = ps.rearrange("p (h d) -> p h d", h=4)
                ro = epipool.tile([P, 4, HD], f32, tag="ro", name="ro")
                cs = epipool.tile([P, 4, HD], f32, tag="cs", name="cs")
                sn = epipool.tile([P, 4, HD], f32, tag="sn", name="sn")
                nc.vector.tensor_tensor(cs[:, :, 0:64], v3[:, :, 0:64], cos4, op=OP.mult)
                nc.vector.tensor_tensor(cs[:, :, 64:128], v3[:, :, 64:128], cos4, op=OP.mult)
                nc.vector.tensor_tensor(sn[:, :, 0:64], v3[:, :, 0:64], sin4, op=OP.mult)
                nc.vector.tensor_tensor(sn[:, :, 64:128], v3[:, :, 64:128], sin4, op=OP.mult)
                nc.vector.tensor_tensor(ro[:, :, 0:64], cs[:, :, 0:64], sn[:, :, 64:128], op=OP.add)
                nc.vector.tensor_sub(ro[:, :, 64:128], cs[:, :, 64:128], sn[:, :, 0:64])
                ss = smallf.tile([P, 4], f32, tag="ss", name="ss")
                sq = epipool.tile([P, 4, HD], f32, tag="cs", name="sq")
                nc.vector.tensor_tensor(sq, ro, ro, op=OP.mult)
                nc.vector.reduce_sum(ss, sq, axis=AX.X)
                rms = smallf.tile([P, 4], f32, tag="rms", name="rms")
                nc.scalar.activation(rms, ss, AF.Sqrt, bias=eps_ap, scale=sqrt_scale)
                rinv = smallf.tile([P, 4], f32, tag="rms", name="rinv")
                nc.vector.reciprocal_approx_fast(rinv, rms)
                for hh in range(4):
                    nc.vector.tensor_scalar_mul(
                        out_bf[:, hh, :], ro[:, hh, :], rinv[:, hh:hh + 1]
                    )

            def pack_transpose(src_bf, dst):
                """src_bf [128, 4, 128] bf16 -> 4 PE transposes -> one copy to
                dst ([128, 4, 128] view)."""
                ptr = psY.tile([P, 512], bf16, tag="Y", name="ptrq")
                for hh in range(4):
                    nc.tensor.transpose(
                        ptr[:, hh * P:(hh + 1) * P], src_bf[:, hh, :], ident
                    )
                nc.vector.tensor_copy(dst, ptr.rearrange("p (s n) -> p s n", s=4))

            def cos_tiles(cap, sap, t0, name):
                cos4 = cspool.tile([P, 4, 64], f32, tag="cs4", name=f"c{name}")
                sin4 = cspool.tile([P, 4, 64], f32, tag="sn4", name=f"s{name}")
                nc.gpsimd.dma_start(out=cos4, in_=bcast4(cap[t0:t0 + P, :]))
                nc.gpsimd.dma_start(out=sin4, in_=bcast4(sap[t0:t0 + P, :]))
                return cos4, sin4

            # ---------------- phase 0k + 1KV: own-key shard K/V + AllGather --
            # tag-shared with xoT below: xkT is dead once K/V proj is done
            xkT = [
                bigpool.tile([P, 4, QROWS], bf16, tag=f"xT{gr}", name=f"xkT{gr}")
                for gr in range(4)
            ]
            for tt in range(NQT):
                x_transpose(xk[tt * P:(tt + 1) * P, :], xkT, tt * P, f"xk{tt}")

            wslk = load_w_slab(wk, 0, "wk")
            wslv = load_w_slab(wv, 0, "wv")
            for tt in range(NQT):
                ps = psS.tile([P, 512], f32, tag="S", name="psk")
                for kt in range(NCT):
                    nc.tensor.matmul(
                        ps,
                        xkT[kt // 4][:, kt % 4, tt * P:(tt + 1) * P],
                        wslk[:, kt, :],
                        start=(kt == 0),
                        stop=(kt == NCT - 1),
                    )
                cos4, sin4 = cos_tiles(cosk, sink, tt * P, f"k{tt}")
                khat = qhpool.tile([P, 4, HD], bf16, tag="qhat", name="khat")
                # fold attn scale into k's rms: 1/sqrt(ss + 128*eps)
                rope_rms(ps, cos4, sin4, khat, eps_k, 1.0)
                drain_pending(1)
                pending.append(
                    lambda khat=khat, tt=tt: pack_transpose(
                        khat, kOwn[:, :, tt * P:(tt + 1) * P]
                    )
                )
                psv = psS.tile([P, 512], f32, tag="S", name="psv")
                for kt in range(NCT):
                    nc.tensor.matmul(
                        psv,
                        xkT[kt // 4][:, kt % 4, tt * P:(tt + 1) * P],
                        wslv[:, kt, :],
                        start=(kt == 0),
                        stop=(kt == NCT - 1),
                    )
                nc.scalar.copy(
                    vOwn[:, tt, :, :], psv.rearrange("p (h d) -> p h d", h=4)
                )
            drain_pending()

            # bounce own K-hat/V shard to DRAM, AllGather within batch group,
            # scatter the gathered shards into the full kT / vA tiles
            nc.sync.dma_start(
                out=kv_in[:, 0:2048], in_=kOwn.rearrange("p a b -> p (a b)")
            )
            nc.sync.dma_start(
                out=kv_in[:, 2048:4096], in_=vOwn.rearrange("p a b c -> p (a b c)")
            )
            nc.gpsimd.collective_compute(
                "AllGather",
                OP.bypass,
                replica_groups=[[0, 1, 2, 3], [4, 5, 6, 7]],
                ins=[kv_in.opt()],
                outs=[kv_out.opt()],
            )
            for r in range(4):
                nc.sync.dma_start(
                    out=kT[:, :, r * QROWS:(r + 1) * QROWS],
                    in_=kv_out[r, :, 0:2048].rearrange("p (a b) -> p a b", a=NKV),
                )
                nc.sync.dma_start(
                    out=vA[:, 4 * r:4 * r + 4, :, :],
                    in_=kv_out[r, :, 2048:4096].rearrange(
                        "p (t h d) -> p t h d", t=4, h=NKV
                    ),
                )

            # ---------------- phase 0a + 1Q: own-row transposes, Q proj ----
            xoT = [
                bigpool.tile([P, 4, QROWS], bf16, tag=f"xT{gr}", name=f"xoT{gr}")
                for gr in range(4)
            ]
            for tt in range(NQT):
                x_transpose(xo[tt * P:(tt + 1) * P, :], xoT, tt * P, f"o{tt}")

            for s in range(4):
                wsl = load_w_slab(wq, s * 512, f"wq{s}")
                for tt in range(NQT):
                    ps = psS.tile([P, 512], f32, tag="S", name="psq")
                    for kt in range(NCT):
                        nc.tensor.matmul(
                            ps,
                            xoT[kt // 4][:, kt % 4, tt * P:(tt + 1) * P],
                            wsl[:, kt, :],
                            start=(kt == 0),
                            stop=(kt == NCT - 1),
                        )
                    cos4, sin4 = cos_tiles(coso, sino, tt * P, f"q{s}{tt}")
                    qhat = qhpool.tile([P, 4, HD], bf16, tag="qhat", name="qhat")
                    rope_rms(ps, cos4, sin4, qhat, eps_q, 1.0 / HD)
                    drain_pending(1)
                    pending.append(
                        lambda qhat=qhat, s=s, tt=tt: pack_transpose(
                            qhat,
                            qT[:, 4 * s:4 * s + 4, (3 - tt) * P:(4 - tt) * P],
                        )
                    )
            drain_pending()

            # ---------------- phase 2: attention (scores-transposed) -------
            # Head-major: for each head, one variable-width matmul per
            # key-tile streams all still-valid query slots at once
            # (kt 0-3 -> N=512 ... kt 12-15 -> N=128). yt/den accumulate all
            # four slots per head in single psum tiles.
            tail_state = []  # (yt_psum, den_psum, h)

            def emit_tail():
                if not tail_state:
                    return
                yt, den, h = tail_state.pop(0)
                rinv = smallf.tile([P, QROWS], f32, tag="rq", name="rqinv")
                nc.vector.reciprocal_approx_fast(rinv, den)
                nc.vector.tensor_tensor(yT[:, h, :], yt, rinv, op=OP.mult)

            def emit_dpv(ent, yt, den, kvh):
                ppt, pkt, pn = ent
                last = pkt == NT - 1
                nc.tensor.matmul(
                    den[:, 0:pn], ones128, ppt[:, 0:pn],
                    start=(pkt == 0), stop=last, skip_group_check=True,
                )
                nc.tensor.matmul(
                    yt[:, 0:pn], vA[:, pkt, kvh, :], ppt[:, 0:pn],
                    start=(pkt == 0), stop=last, skip_group_check=True,
                )

            def load_wo_slab(s3):
                w3 = wopool.tile([P, NCT, 512], bf16, tag="wo3", name=f"wo{s3}")
                for gr in range(4):
                    nc.gpsimd.dma_start(
                        out=w3[:, 4 * gr:4 * gr + 4, :],
                        in_=wo[:, s3 * 512:s3 * 512 + 512].rearrange(
                            "(a p) n -> p a n", p=P
                        )[:, 4 * gr:4 * gr + 4, :],
                    )
                return w3

            # prefetch the first two wo slabs; their DMAs run under phase 2
            w3s = {0: load_wo_slab(0), 1: load_wo_slab(1)}

            for h in range(NH):
                kvh = h // (NH // NKV)
                yt = psY.tile([P, QROWS], f32, tag="Y", name="yt")
                den = psD.tile([P, QROWS], f32, tag="D", name="den")
                dq = []  # exp'd tiles awaiting den/PV (2-deep pipeline)
                for kt in range(NT):
                    # q-slot columns are stored high-slot-first, so the
                    # still-valid slots for key tile kt are columns [0, n)
                    n = QROWS - (kt // 4) * P
                    S = psS.tile([P, 512], f32, tag="S", name="Sb")
                    nc.tensor.matmul(
                        S[:, 0:n],
                        kT[:, kvh, kt * P:(kt + 1) * P],
                        qT[:, h, 0:n],
                        start=True,
                        stop=False,
                        skip_group_check=True,
                    )
                    # causal mask for the diagonal slot (last 128 valid cols),
                    # accumulated on the PE via an identity matmul
                    nc.tensor.matmul(
                        S[:, n - P:n], ident, qmask[:, kt // 4, kt % 4, :],
                        start=False, stop=True, skip_group_check=True,
                    )
                    if kt == 0 and tail_state:
                        emit_tail()
                    # attn scale already folded into k's rms normalization
                    pt = ptpool.tile([P, 512], bf16, tag="pt", name="pt")
                    nc.scalar.activation(pt[:, 0:n], S[:, 0:n], AF.Exp, scale=1.0)
                    dq.append((pt, kt, n))
                    if len(dq) > 2:
                        emit_dpv(dq.pop(0), yt, den, kvh)
                while dq:
                    emit_dpv(dq.pop(0), yt, den, kvh)
                tail_state.append((yt, den, h))
            emit_tail()

            # ---------------- phase 3: output projection ----------------
            for s3 in range(4):
                w3 = w3s.pop(s3)
                if s3 + 2 < 4:
                    w3s[s3 + 2] = load_wo_slab(s3 + 2)
                for qt in range(4):
                    ps = psS.tile([P, 512], f32, tag="S", name="ps3")
                    for ct in range(NCT):
                        nc.tensor.matmul(
                            ps,
                            yT[:, ct, (3 - qt) * P:(4 - qt) * P],
                            w3[:, ct, :],
                            start=(ct == 0),
                            stop=(ct == NCT - 1),
                        )
                    ot = outpool.tile([P, 512], f32, tag="ot", name="ot")
                    nc.vector.tensor_copy(ot, ps)
                    nc.sync.dma_start(
                        out=yo[qt * P:(qt + 1) * P, s3 * 512:(s3 + 1) * 512],
                        in_=ot,
                    )

    nc.compile()
    return nc


def _get_nc():
    if "nc" not in _CACHE:
        _CACHE["nc"] = _build()
    return _CACHE["nc"]


def _in_maps(x, cosr, sinr, wq, wk, wv, wo):
    maps = []
    for core in range(8):
        b, g = core // 4, core % 4
        rows = _rows(g)
        ksl = slice(g * QROWS, (g + 1) * QROWS)
        maps.append({
            "xo": np.ascontiguousarray(x[b][rows]),
            "xk": np.ascontiguousarray(x[b][ksl]),
            "coso": np.ascontiguousarray(cosr[rows]),
            "sino": np.ascontiguousarray(sinr[rows]),
            "cosk": np.ascontiguousarray(cosr[ksl]),
            "sink": np.ascontiguousarray(sinr[ksl]),
            "wq": wq, "wk": wk, "wv": wv, "wo": wo,
            "qm": _qmask_t(g),
        })
    return maps


def kernel(x, cos, sin, wq, wk, wv, wo):
    from concourse.bass_utils import run_bass_kernel_spmd

    x = np.ascontiguousarray(np.asarray(x, np.float32))
    cosr = np.ascontiguousarray(np.asarray(cos, np.float32).reshape(T, HD // 2))
    sinr = np.ascontiguousarray(np.asarray(sin, np.float32).reshape(T, HD // 2))
    wq = np.ascontiguousarray(np.asarray(wq, np.float32))
    wk = np.ascontiguousarray(np.asarray(wk, np.float32))
    wv = np.ascontiguousarray(np.asarray(wv, np.float32))
    wo = np.ascontiguousarray(np.asarray(wo, np.float32))

    nc = _get_nc()
    maps = _in_maps(x, cosr, sinr, wq, wk, wv, wo)
    _CACHE["in_maps"] = maps
    res = run_bass_kernel_spmd(nc, maps, list(range(8)))
    y = np.empty((B, T, C), np.float32)
    for core in range(8):
        b, g = core // 4, core % 4
        y[b][_rows(g)] = res.results[core]["yo"]
    return y


# revision 4
# speedup vs baseline: 1.3235x; 1.1192x over previous
"""Causal self-attention (GQA, rope, qk-rmsnorm) Trainium2 kernel, 8 NeuronCores.

Sharding: core = (b, g), b = core // 4 (batch), g = core % 4.
Each core handles query row-chunks {g, 4+g, 8+g, 12+g} (128 rows each) of its
batch: computes Q for those 512 rows, K/V for ONLY its contiguous 512-key
shard [512g, 512(g+1)), then AllGathers the rope+rms'd K-hat and raw V
across the 4 cores of its batch (replica groups [[0..3],[4..7]]) so every
core has the full 2048 keys.  Attention for all 16 heads and the 512-row
slice of the output projection follow as before.  Host gathers row slices.
The program is identical on all cores (SPMD); all per-core variation comes
through the input shards.

Slot c (c = 0..3) covers query chunk 4c+g with keys [0, 512*(c+1)) — uniform
across cores; causal masking inside the last 512 keys comes from a
host-provided additive mask shard.
"""

import sys

if "/opt/trn_rl_repo" not in sys.path:
    sys.path.insert(0, "/opt/trn_rl_repo")

import numpy as np

B, T, C = 2, 2048, 2048
NH, NKV = 16, 4
HD = C // NH  # 128
P = 128
NT = T // P            # 16 token tiles per batch
NCT = C // P           # 16 contraction tiles
QROWS = 512            # own query rows per core
NQT = QROWS // P       # 4 own token tiles
KLEN = [512, 1024, 1536, 2048]   # keys per slot
SCALE = 1.0 / float(np.sqrt(HD))
EPS = float(np.finfo(np.float32).eps)
NEG = -1.0e9

_CACHE = {}


def _chunks(g):
    return [g, 4 + g, 8 + g, 12 + g]


def _rows(g):
    return np.concatenate([np.arange(ch * P, (ch + 1) * P) for ch in _chunks(g)])


def _qmask_t(g):
    """Additive mask, transposed layout: (slot c, sub s, k_in_sub i, q j).

    For slot c the score tile is S^T[k, q] with k in [0, KLEN[c]) and q the
    128 rows of chunk 4c+g. Only keys in the last 512 of the slot can be
    invalid; mask[c, s, i, j] = 0 if key (KLEN[c]-512 + s*128 + i) <= query
    (128*(4c+g) + j) else NEG.
    """
    m = np.zeros((4, 4, P, P), np.float32)
    for c in range(4):
        k0 = KLEN[c] - 512
        r0 = (4 * c + g) * P
        k = k0 + np.arange(512)[:, None]          # (512, 1)
        q = r0 + np.arange(P)[None, :]            # (1, 128)
        m[c] = np.where(k <= q, 0.0, NEG).reshape(4, P, P)
    return m


def _build():
    import concourse.bacc as bacc
    import concourse.bass as bass
    import concourse.mybir as mybir
    import concourse.tile as tile
    from concourse.masks import make_identity

    f32 = mybir.dt.float32
    bf16 = mybir.dt.bfloat16
    AF = mybir.ActivationFunctionType
    OP = mybir.AluOpType
    AX = mybir.AxisListType

    nc = bacc.Bacc("TRN2", target_bir_lowering=False, debug=False, num_devices=8)

    xo = nc.dram_tensor("xo", [QROWS, C], bf16, kind="ExternalInput").ap()
    xk = nc.dram_tensor("xk", [QROWS, C], bf16, kind="ExternalInput").ap()
    coso = nc.dram_tensor("coso", [QROWS, HD // 2], f32, kind="ExternalInput").ap()
    sino = nc.dram_tensor("sino", [QROWS, HD // 2], f32, kind="ExternalInput").ap()
    cosk = nc.dram_tensor("cosk", [QROWS, HD // 2], f32, kind="ExternalInput").ap()
    sink = nc.dram_tensor("sink", [QROWS, HD // 2], f32, kind="ExternalInput").ap()
    wq = nc.dram_tensor("wq", [C, C], bf16, kind="ExternalInput").ap()
    wk = nc.dram_tensor("wk", [C, NKV * HD], bf16, kind="ExternalInput").ap()
    wv = nc.dram_tensor("wv", [C, NKV * HD], bf16, kind="ExternalInput").ap()
    wo = nc.dram_tensor("wo", [C, C], bf16, kind="ExternalInput").ap()
    qm = nc.dram_tensor("qm", [4, 4, P, P], bf16, kind="ExternalInput").ap()
    yo = nc.dram_tensor("yo", [QROWS, C], f32, kind="ExternalOutput").ap()

    def bcast4(ap2d):
        # [128, 64] -> [128, 4, 64] with middle step 0 (replicate across heads)
        return bass.AP(
            tensor=ap2d.tensor,
            offset=ap2d.offset,
            ap=[ap2d.ap[0], [0, 4], ap2d.ap[1]],
        )

    with tile.TileContext(nc) as tc:
        with (
            tc.tile_pool(name="singles", bufs=1) as singles,
            tc.tile_pool(name="big", bufs=1) as bigpool,
            tc.tile_pool(name="xb", bufs=2) as xbpool,
            tc.tile_pool(name="wsl", bufs=2) as wslpool,
            tc.tile_pool(name="cs", bufs=6) as cspool,
            tc.tile_pool(name="epi", bufs=2) as epipool,
            tc.tile_pool(name="qh", bufs=3) as qhpool,
            tc.tile_pool(name="wo3", bufs=2) as wopool,
            tc.tile_pool(name="pt", bufs=4) as ptpool,
            tc.tile_pool(name="smallf", bufs=2) as smallf,
            tc.tile_pool(name="outs", bufs=2) as outpool,
            tc.tile_pool(name="dramb", bufs=1, space="DRAM") as drampool,
            tc.tile_pool(name="psS", bufs=3, space="PSUM") as psS,
            tc.tile_pool(name="psY", bufs=3, space="PSUM") as psY,
            tc.tile_pool(name="psD", bufs=2, space="PSUM") as psD,
        ):
            ident = singles.tile([P, P], bf16)
            make_identity(nc, ident)
            identf = singles.tile([P, P], f32)
            make_identity(nc, identf)
            ones128 = singles.tile([P, P], bf16)
            nc.vector.memset(ones128, 1.0)
            eps_q = singles.tile([P, 1], f32)
            nc.vector.memset(eps_q, EPS)
            eps_k = singles.tile([P, 1], f32)
            nc.vector.memset(eps_k, HD * EPS)

            # persistent big SBUF tensors
            qT = bigpool.tile([P, NH, QROWS], bf16, tag="qT")      # [d, h, q]
            kT = bigpool.tile([P, NKV, T], bf16, tag="kT")         # [d, kvh, k]
            vA = bigpool.tile([P, NT, NKV, HD], bf16, tag="vA")    # [ktok, tt, kvh, d]
            kOwn = bigpool.tile([P, NKV, QROWS], bf16, tag="kOwn")  # own shard
            vOwn = bigpool.tile([P, NQT, NKV, HD], bf16, tag="vOwn")
            yT = bigpool.tile([P, NCT, QROWS], bf16, tag="yT")     # [d, ct, q]
            qmask = singles.tile([P, 4, 4, P], bf16)               # [ki, c, sub, q]
            nc.gpsimd.dma_start(out=qmask, in_=qm.rearrange("c s i j -> i c s j"))

            # DRAM bounce buffers for the K/V AllGather (bf16)
            kv_in = drampool.tile([P, 4096], bf16)                 # 1 MB
            kv_out = drampool.tile([4, P, 4096], bf16)             # 4 MB

            # ---------------- helpers ----------------
            def x_transpose(src_ap, dst4, dst_col, name):
                """Load one 128-row strip of x (f32 dram) via HWDGE, transpose
                (f32) into the 4 [P, 4, width] bf16 dst tiles at dst_col."""
                for h2 in range(2):
                    xb = xbpool.tile([P, 8 * P], bf16, tag="xb", name=f"xb{name}{h2}")
                    nc.sync.dma_start(
                        out=xb, in_=src_ap[:, h2 * 1024:(h2 + 1) * 1024]
                    )
                    for gr in (2 * h2, 2 * h2 + 1):
                        ptr = psY.tile([P, 512], bf16, tag="Y", name=f"ptr{name}{gr}")
                        for j in range(4):
                            ctl = 4 * gr + j - 8 * h2
                            nc.tensor.transpose(
                                ptr[:, j * P:(j + 1) * P],
                                xb[:, ctl * P:(ctl + 1) * P],
                                ident,
                            )
                        nc.scalar.copy(
                            dst4[gr][:, :, dst_col:dst_col + P],
                            ptr.rearrange("p (s n) -> p s n", s=4),
                        )

            def load_w_slab(w_ap, col0, name):
                """One 512-col slab of a weight, as [128, 16, 512] bf16."""
                wsl = wslpool.tile([P, NCT, 512], bf16, tag="wsl", name=name)
                for gr in range(4):
                    nc.gpsimd.dma_start(
                        out=wsl[:, 4 * gr:4 * gr + 4, :],
                        in_=w_ap[:, col0:col0 + 512].rearrange(
                            "(a p) n -> p a n", p=P
                        )[:, 4 * gr:4 * gr + 4, :],
                    )
                return wsl

            pending = []  # delayed PE transpose packs (2-deep pipeline)

            def drain_pending(keep=0):
                while len(pending) > keep:
                    pending.pop(0)()

            def rope_rms(ps, cos4, sin4, out_bf, eps_ap, sqrt_scale):
                """ps: [128, 512] psum f32 (4 heads). Writes normalized bf16
                rope output to out_bf [128, 4, 128]."""
                v3 = ps.rearrange("p (h d) -> p h d", h=4)
                ro = epipool.tile([P, 4, HD], f32, tag="ro", name="ro")
                cs = epipool.tile([P, 4, HD], f32, tag="cs", name="cs")
                sn = epipool.tile([P, 4, HD], f32, tag="sn", name="sn")
                nc.vector.tensor_tensor(cs[:, :, 0:64], v3[:, :, 0:64], cos4, op=OP.mult)
                nc.vector.tensor_tensor(cs[:, :, 64:128], v3[:, :, 64:128], cos4, op=OP.mult)
                nc.vector.tensor_tensor(sn[:, :, 0:64], v3[:, :, 0:64], sin4, op=OP.mult)
                nc.vector.tensor_tensor(sn[:, :, 64:128], v3[:, :, 64:128], sin4, op=OP.mult)
                nc.vector.tensor_tensor(ro[:, :, 0:64], cs[:, :, 0:64], sn[:, :, 64:128], op=OP.add)
                nc.vector.tensor_sub(ro[:, :, 64:128], cs[:, :, 64:128], sn[:, :, 0:64])
                ss = smallf.tile([P, 4], f32, tag="ss", name="ss")
                sq = epipool.tile([P, 4, HD], f32, tag="cs", name="sq")
                nc.vector.tensor_tensor(sq, ro, ro, op=OP.mult)
                nc.vector.reduce_sum(ss, sq, axis=AX.X)
                rms = smallf.tile([P, 4], f32, tag="rms", name="rms")
                nc.scalar.activation(rms, ss, AF.Sqrt, bias=eps_ap, scale=sqrt_scale)
                rinv = smallf.tile([P, 4], f32, tag="rms", name="rinv")
                nc.vector.reciprocal_approx_fast(rinv, rms)
                for hh in range(4):
                    nc.vector.tensor_scalar_mul(
                        out_bf[:, hh, :], ro[:, hh, :], rinv[:, hh:hh + 1]
                    )

            def pack_transpose(src_bf, dst):
                """src_bf [128, 4, 128] bf16 -> 4 PE transposes -> one copy to
                dst ([128, 4, 128] view)."""
                ptr = psY.tile([P, 512], bf16, tag="Y", name="ptrq")
                for hh in range(4):
                    nc.tensor.transpose(
                        ptr[:, hh * P:(hh + 1) * P], src_bf[:, hh, :], ident
                    )
                nc.vector.tensor_copy(dst, ptr.rearrange("p (s n) -> p s n", s=4))

            def cos_tiles(cap, sap, t0, name):
                cos4 = cspool.tile([P, 4, 64], f32, tag="cs4", name=f"c{name}")
                sin4 = cspool.tile([P, 4, 64], f32, tag="sn4", name=f"s{name}")
                nc.gpsimd.dma_start(out=cos4, in_=bcast4(cap[t0:t0 + P, :]))
                nc.gpsimd.dma_start(out=sin4, in_=bcast4(sap[t0:t0 + P, :]))
                return cos4, sin4

            # ---------------- phase 0k + 1KV: own-key shard K/V + AllGather --
            # tag-shared with xoT below: xkT is dead once K/V proj is done
            xkT = [
                bigpool.tile([P, 4, QROWS], bf16, tag=f"xT{gr}", name=f"xkT{gr}")
                for gr in range(4)
            ]
            for tt in range(NQT):
                x_transpose(xk[tt * P:(tt + 1) * P, :], xkT, tt * P, f"xk{tt}")

            wslk = load_w_slab(wk, 0, "wk")
            wslv = load_w_slab(wv, 0, "wv")
            for tt in range(NQT):
                ps = psS.tile([P, 512], f32, tag="S", name="psk")
                for kt in range(NCT):
                    nc.tensor.matmul(
                        ps,
                        xkT[kt // 4][:, kt % 4, tt * P:(tt + 1) * P],
                        wslk[:, kt, :],
                        start=(kt == 0),
                        stop=(kt == NCT - 1),
                    )
                cos4, sin4 = cos_tiles(cosk, sink, tt * P, f"k{tt}")
                khat = qhpool.tile([P, 4, HD], bf16, tag="qhat", name="khat")
                # fold attn scale into k's rms: 1/sqrt(ss + 128*eps)
                rope_rms(ps, cos4, sin4, khat, eps_k, 1.0)
                drain_pending(1)
                pending.append(
                    lambda khat=khat, tt=tt: pack_transpose(
                        khat, kOwn[:, :, tt * P:(tt + 1) * P]
                    )
                )
                psv = psS.tile([P, 512], f32, tag="S", name="psv")
                for kt in range(NCT):
                    nc.tensor.matmul(
                        psv,
                        xkT[kt // 4][:, kt % 4, tt * P:(tt + 1) * P],
                        wslv[:, kt, :],
                        start=(kt == 0),
                        stop=(kt == NCT - 1),
                    )
                nc.scalar.copy(
                    vOwn[:, tt, :, :], psv.rearrange("p (h d) -> p h d", h=4)
                )
            drain_pending()

            # bounce own K-hat/V shard to DRAM, AllGather within batch group,
            # scatter the gathered shards into the full kT / vA tiles
            nc.sync.dma_start(
                out=kv_in[:, 0:2048], in_=kOwn.rearrange("p a b -> p (a b)")
            )
            nc.sync.dma_start(
                out=kv_in[:, 2048:4096], in_=vOwn.rearrange("p a b c -> p (a b c)")
            )
            nc.gpsimd.collective_compute(
                "AllGather",
                OP.bypass,
                replica_groups=[[0, 1, 2, 3], [4, 5, 6, 7]],
                ins=[kv_in.opt()],
                outs=[kv_out.opt()],
            )
            for r in range(4):
                nc.sync.dma_start(
                    out=kT[:, :, r * QROWS:(r + 1) * QROWS],
                    in_=kv_out[r, :, 0:2048].rearrange("p (a b) -> p a b", a=NKV),
                )
                nc.sync.dma_start(
                    out=vA[:, 4 * r:4 * r + 4, :, :],
                    in_=kv_out[r, :, 2048:4096].rearrange(
                        "p (t h d) -> p t h d", t=4, h=NKV
                    ),
                )

            # ---------------- phase 0a + 1Q: own-row transposes, Q proj ----
            xoT = [
                bigpool.tile([P, 4, QROWS], bf16, tag=f"xT{gr}", name=f"xoT{gr}")
                for gr in range(4)
            ]
            for tt in range(NQT):
                x_transpose(xo[tt * P:(tt + 1) * P, :], xoT, tt * P, f"o{tt}")

            qcs = {tt: cos_tiles(coso, sino, tt * P, f"q{tt}") for tt in range(NQT)}
            for s in range(4):
                wsl = load_w_slab(wq, s * 512, f"wq{s}")
                for tt in range(NQT):
                    ps = psS.tile([P, 512], f32, tag="S", name="psq")
                    for kt in range(NCT):
                        nc.tensor.matmul(
                            ps,
                            xoT[kt // 4][:, kt % 4, tt * P:(tt + 1) * P],
                            wsl[:, kt, :],
                            start=(kt == 0),
                            stop=(kt == NCT - 1),
                        )
                    cos4, sin4 = qcs[tt]
                    qhat = qhpool.tile([P, 4, HD], bf16, tag="qhat", name="qhat")
                    rope_rms(ps, cos4, sin4, qhat, eps_q, 1.0 / HD)
                    drain_pending(1)
                    pending.append(
                        lambda qhat=qhat, s=s, tt=tt: pack_transpose(
                            qhat,
                            qT[:, 4 * s:4 * s + 4, (3 - tt) * P:(4 - tt) * P],
                        )
                    )
            drain_pending()

            # ---------------- phase 2: attention (scores-transposed) -------
            # Head-major: for each head, one variable-width matmul per
            # key-tile streams all still-valid query slots at once
            # (kt 0-3 -> N=512 ... kt 12-15 -> N=128). yt/den accumulate all
            # four slots per head in single psum tiles.
            tail_state = []  # (yt_psum, den_psum, h)

            def emit_tail():
                if not tail_state:
                    return
                yt, den, h = tail_state.pop(0)
                rinv = smallf.tile([P, QROWS], f32, tag="rq", name="rqinv")
                nc.vector.reciprocal_approx_fast(rinv, den)
                nc.vector.tensor_tensor(yT[:, h, :], yt, rinv, op=OP.mult)

            def emit_dpv(ent, yt, den, kvh):
                ppt, pkt, pn = ent
                last = pkt == NT - 1
                nc.tensor.matmul(
                    den[:, 0:pn], ones128, ppt[:, 0:pn],
                    start=(pkt == 0), stop=last, skip_group_check=True,
                )
                nc.tensor.matmul(
                    yt[:, 0:pn], vA[:, pkt, kvh, :], ppt[:, 0:pn],
                    start=(pkt == 0), stop=last, skip_group_check=True,
                )

            def load_wo_slab(s3):
                w3 = wopool.tile([P, NCT, 512], bf16, tag="wo3", name=f"wo{s3}")
                for gr in range(4):
                    nc.gpsimd.dma_start(
                        out=w3[:, 4 * gr:4 * gr + 4, :],
                        in_=wo[:, s3 * 512:s3 * 512 + 512].rearrange(
                            "(a p) n -> p a n", p=P
                        )[:, 4 * gr:4 * gr + 4, :],
                    )
                return w3

            # prefetch the first two wo slabs; their DMAs run under phase 2
            w3s = {0: load_wo_slab(0), 1: load_wo_slab(1)}

            for h in range(NH):
                kvh = h // (NH // NKV)
                yt = psY.tile([P, QROWS], f32, tag="Y", name="yt")
                den = psD.tile([P, QROWS], f32, tag="D", name="den")
                dq = []  # exp'd tiles awaiting den/PV (2-deep pipeline)
                for kt in range(NT):
                    # q-slot columns are stored high-slot-first, so the
                    # still-valid slots for key tile kt are columns [0, n)
                    n = QROWS - (kt // 4) * P
                    S = psS.tile([P, 512], f32, tag="S", name="Sb")
                    nc.tensor.matmul(
                        S[:, 0:n],
                        kT[:, kvh, kt * P:(kt + 1) * P],
                        qT[:, h, 0:n],
                        start=True,
                        stop=False,
                        skip_group_check=True,
                    )
                    # causal mask for the diagonal slot (last 128 valid cols),
                    # accumulated on the PE via an identity matmul
                    nc.tensor.matmul(
                        S[:, n - P:n], ident, qmask[:, kt // 4, kt % 4, :],
                        start=False, stop=True, skip_group_check=True,
                    )
                    if kt == 0 and tail_state:
                        emit_tail()
                    # attn scale already folded into k's rms normalization
                    pt = ptpool.tile([P, 512], bf16, tag="pt", name="pt")
                    nc.scalar.activation(pt[:, 0:n], S[:, 0:n], AF.Exp, scale=1.0)
                    dq.append((pt, kt, n))
                    if len(dq) > 2:
                        emit_dpv(dq.pop(0), yt, den, kvh)
                while dq:
                    emit_dpv(dq.pop(0), yt, den, kvh)
                tail_state.append((yt, den, h))
            emit_tail()

            # ---------------- phase 3: output projection ----------------
            for s3 in range(4):
                w3 = w3s.pop(s3)
                if s3 + 2 < 4:
                    w3s[s3 + 2] = load_wo_slab(s3 + 2)
                for qt in range(4):
                    ps = psS.tile([P, 512], f32, tag="S", name="ps3")
                    for ct in range(NCT):
                        nc.tensor.matmul(
                            ps,
                            yT[:, ct, (3 - qt) * P:(4 - qt) * P],
                            w3[:, ct, :],
                            start=(ct == 0),
                            stop=(ct == NCT - 1),
                        )
                    ot = outpool.tile([P, 512], f32, tag="ot", name="ot")
                    nc.vector.tensor_copy(ot, ps)
                    nc.sync.dma_start(
                        out=yo[qt * P:(qt + 1) * P, s3 * 512:(s3 + 1) * 512],
                        in_=ot,
                    )

    nc.compile()
    return nc


def _get_nc():
    if "nc" not in _CACHE:
        _CACHE["nc"] = _build()
    return _CACHE["nc"]


def _in_maps(x, cosr, sinr, wq, wk, wv, wo):
    import ml_dtypes
    bf = ml_dtypes.bfloat16
    x = x.astype(bf)
    wq, wk, wv, wo = (a.astype(bf) for a in (wq, wk, wv, wo))
    maps = []
    for core in range(8):
        b, g = core // 4, core % 4
        rows = _rows(g)
        ksl = slice(g * QROWS, (g + 1) * QROWS)
        maps.append({
            "xo": np.ascontiguousarray(x[b][rows]),
            "xk": np.ascontiguousarray(x[b][ksl]),
            "coso": np.ascontiguousarray(cosr[rows]),
            "sino": np.ascontiguousarray(sinr[rows]),
            "cosk": np.ascontiguousarray(cosr[ksl]),
            "sink": np.ascontiguousarray(sinr[ksl]),
            "wq": wq, "wk": wk, "wv": wv, "wo": wo,
            "qm": _qmask_t(g).astype(ml_dtypes.bfloat16),
        })
    return maps


def kernel(x, cos, sin, wq, wk, wv, wo):
    from concourse.bass_utils import run_bass_kernel_spmd

    x = np.ascontiguousarray(np.asarray(x, np.float32))
    cosr = np.ascontiguousarray(np.asarray(cos, np.float32).reshape(T, HD // 2))
    sinr = np.ascontiguousarray(np.asarray(sin, np.float32).reshape(T, HD // 2))
    wq = np.ascontiguousarray(np.asarray(wq, np.float32))
    wk = np.ascontiguousarray(np.asarray(wk, np.float32))
    wv = np.ascontiguousarray(np.asarray(wv, np.float32))
    wo = np.ascontiguousarray(np.asarray(wo, np.float32))

    nc = _get_nc()
    maps = _in_maps(x, cosr, sinr, wq, wk, wv, wo)
    _CACHE["in_maps"] = maps
    res = run_bass_kernel_spmd(nc, maps, list(range(8)))
    y = np.empty((B, T, C), np.float32)
    for core in range(8):
        b, g = core // 4, core % 4
        y[b][_rows(g)] = res.results[core]["yo"]
    return y


# revision 5
# speedup vs baseline: 1.3444x; 1.0158x over previous
"""Causal self-attention (GQA, rope, qk-rmsnorm) Trainium2 kernel, 8 NeuronCores.

Sharding: core = (b, g), b = core // 4 (batch), g = core % 4.
Each core handles query row-chunks {g, 4+g, 8+g, 12+g} (128 rows each) of its
batch: computes Q for those 512 rows, K/V for ONLY its contiguous 512-key
shard [512g, 512(g+1)), then AllGathers the rope+rms'd K-hat and raw V
across the 4 cores of its batch (replica groups [[0..3],[4..7]]) so every
core has the full 2048 keys.  Attention for all 16 heads and the 512-row
slice of the output projection follow as before.  Host gathers row slices.
The program is identical on all cores (SPMD); all per-core variation comes
through the input shards.

Slot c (c = 0..3) covers query chunk 4c+g with keys [0, 512*(c+1)) — uniform
across cores; causal masking inside the last 512 keys comes from a
host-provided additive mask shard.
"""

import sys

if "/opt/trn_rl_repo" not in sys.path:
    sys.path.insert(0, "/opt/trn_rl_repo")

import numpy as np

B, T, C = 2, 2048, 2048
NH, NKV = 16, 4
HD = C // NH  # 128
P = 128
NT = T // P            # 16 token tiles per batch
NCT = C // P           # 16 contraction tiles
QROWS = 512            # own query rows per core
NQT = QROWS // P       # 4 own token tiles
KLEN = [512, 1024, 1536, 2048]   # keys per slot
SCALE = 1.0 / float(np.sqrt(HD))
EPS = float(np.finfo(np.float32).eps)
NEG = -1.0e9

_CACHE = {}


def _chunks(g):
    return [g, 4 + g, 8 + g, 12 + g]


def _rows(g):
    return np.concatenate([np.arange(ch * P, (ch + 1) * P) for ch in _chunks(g)])


def _qmask_t(g):
    """Additive mask, transposed layout: (slot c, sub s, k_in_sub i, q j).

    For slot c the score tile is S^T[k, q] with k in [0, KLEN[c]) and q the
    128 rows of chunk 4c+g. Only keys in the last 512 of the slot can be
    invalid; mask[c, s, i, j] = 0 if key (KLEN[c]-512 + s*128 + i) <= query
    (128*(4c+g) + j) else NEG.
    """
    m = np.zeros((4, 4, P, P), np.float32)
    for c in range(4):
        k0 = KLEN[c] - 512
        r0 = (4 * c + g) * P
        k = k0 + np.arange(512)[:, None]          # (512, 1)
        q = r0 + np.arange(P)[None, :]            # (1, 128)
        m[c] = np.where(k <= q, 1.0, 0.0).reshape(4, P, P)
    return m


def _build():
    import concourse.bacc as bacc
    import concourse.bass as bass
    import concourse.mybir as mybir
    import concourse.tile as tile
    from concourse.masks import make_identity

    f32 = mybir.dt.float32
    bf16 = mybir.dt.bfloat16
    AF = mybir.ActivationFunctionType
    OP = mybir.AluOpType
    AX = mybir.AxisListType

    nc = bacc.Bacc("TRN2", target_bir_lowering=False, debug=False, num_devices=8)

    xo = nc.dram_tensor("xo", [QROWS, C], bf16, kind="ExternalInput").ap()
    xk = nc.dram_tensor("xk", [QROWS, C], bf16, kind="ExternalInput").ap()
    coso = nc.dram_tensor("coso", [QROWS, HD // 2], f32, kind="ExternalInput").ap()
    sino = nc.dram_tensor("sino", [QROWS, HD // 2], f32, kind="ExternalInput").ap()
    cosk = nc.dram_tensor("cosk", [QROWS, HD // 2], f32, kind="ExternalInput").ap()
    sink = nc.dram_tensor("sink", [QROWS, HD // 2], f32, kind="ExternalInput").ap()
    wq = nc.dram_tensor("wq", [C, C], bf16, kind="ExternalInput").ap()
    wk = nc.dram_tensor("wk", [C, NKV * HD], bf16, kind="ExternalInput").ap()
    wv = nc.dram_tensor("wv", [C, NKV * HD], bf16, kind="ExternalInput").ap()
    wo = nc.dram_tensor("wo", [C, C], bf16, kind="ExternalInput").ap()
    qm = nc.dram_tensor("qm", [4, 4, P, P], bf16, kind="ExternalInput").ap()
    yo = nc.dram_tensor("yo", [QROWS, C], f32, kind="ExternalOutput").ap()

    def bcast_sbuf(ap, pos, n):
        # insert a 0-stride dim of size n at position pos (broadcast view)
        a = [list(d) for d in ap.ap]
        a.insert(pos, [0, n])
        return bass.AP(tensor=ap.tensor, offset=ap.offset, ap=a)

    def bcast4(ap2d):
        # [128, 64] -> [128, 4, 64] with middle step 0 (replicate across heads)
        return bass.AP(
            tensor=ap2d.tensor,
            offset=ap2d.offset,
            ap=[ap2d.ap[0], [0, 4], ap2d.ap[1]],
        )

    with tile.TileContext(nc) as tc:
        with (
            tc.tile_pool(name="singles", bufs=1) as singles,
            tc.tile_pool(name="big", bufs=1) as bigpool,
            tc.tile_pool(name="xb", bufs=3) as xbpool,
            tc.tile_pool(name="wsl", bufs=2) as wslpool,
            tc.tile_pool(name="cs", bufs=6) as cspool,
            tc.tile_pool(name="epi", bufs=2) as epipool,
            tc.tile_pool(name="qh", bufs=3) as qhpool,
            tc.tile_pool(name="wo3", bufs=2) as wopool,
            tc.tile_pool(name="pt", bufs=4) as ptpool,
            tc.tile_pool(name="smallf", bufs=2) as smallf,
            tc.tile_pool(name="outs", bufs=2) as outpool,
            tc.tile_pool(name="dramb", bufs=1, space="DRAM") as drampool,
            tc.tile_pool(name="psS", bufs=3, space="PSUM") as psS,
            tc.tile_pool(name="psY", bufs=3, space="PSUM") as psY,
            tc.tile_pool(name="psD", bufs=2, space="PSUM") as psD,
        ):
            ident = singles.tile([P, P], bf16)
            make_identity(nc, ident)
            identf = singles.tile([P, P], f32)
            make_identity(nc, identf)
            ones128 = singles.tile([P, P], bf16)
            nc.vector.memset(ones128, 1.0)
            eps_q = singles.tile([P, 1], f32)
            nc.vector.memset(eps_q, EPS)
            eps_k = singles.tile([P, 1], f32)
            nc.vector.memset(eps_k, HD * EPS)

            # persistent big SBUF tensors
            qT = bigpool.tile([P, NH, QROWS], bf16, tag="qT")      # [d, h, q]
            kT = bigpool.tile([P, NKV, T], bf16, tag="kT")         # [d, kvh, k]
            vA = bigpool.tile([P, NT, NKV, HD], bf16, tag="vA")    # [ktok, tt, kvh, d]
            kOwn = bigpool.tile([P, NKV, QROWS], bf16, tag="kOwn")  # own shard
            vOwn = bigpool.tile([P, NQT, NKV, HD], bf16, tag="vOwn")
            yT = bigpool.tile([P, NCT, QROWS], bf16, tag="yT")     # [d, ct, q]
            qmask = singles.tile([P, 4, 4, P], bf16)               # [ki, c, sub, q]
            nc.gpsimd.dma_start(out=qmask, in_=qm.rearrange("c s i j -> i c s j"))

            # DRAM bounce buffers for the K/V AllGather (bf16)
            kv_in = drampool.tile([P, 4096], bf16)                 # 1 MB
            kv_out = drampool.tile([4, P, 4096], bf16)             # 4 MB

            # ---------------- helpers ----------------
            def x_transpose(src_ap, dst4, dst_col, name):
                """Load one 128-row strip of x (f32 dram) via HWDGE, transpose
                (f32) into the 4 [P, 4, width] bf16 dst tiles at dst_col."""
                for h2 in range(2):
                    xb = xbpool.tile([P, 8 * P], bf16, tag="xb", name=f"xb{name}{h2}")
                    nc.sync.dma_start(
                        out=xb, in_=src_ap[:, h2 * 1024:(h2 + 1) * 1024]
                    )
                    for gr in (2 * h2, 2 * h2 + 1):
                        ptr = psY.tile([P, 512], bf16, tag="Y", name=f"ptr{name}{gr}")
                        for j in range(4):
                            ctl = 4 * gr + j - 8 * h2
                            nc.tensor.transpose(
                                ptr[:, j * P:(j + 1) * P],
                                xb[:, ctl * P:(ctl + 1) * P],
                                ident,
                            )
                        nc.scalar.copy(
                            dst4[gr][:, :, dst_col:dst_col + P],
                            ptr.rearrange("p (s n) -> p s n", s=4),
                        )

            def load_w_slab(w_ap, col0, name):
                """One 512-col slab of a weight, as [128, 16, 512] bf16."""
                wsl = wslpool.tile([P, NCT, 512], bf16, tag="wsl", name=name)
                for gr in range(4):
                    nc.gpsimd.dma_start(
                        out=wsl[:, 4 * gr:4 * gr + 4, :],
                        in_=w_ap[:, col0:col0 + 512].rearrange(
                            "(a p) n -> p a n", p=P
                        )[:, 4 * gr:4 * gr + 4, :],
                    )
                return wsl

            pending = []  # delayed PE transpose packs (2-deep pipeline)

            def drain_pending(keep=0):
                while len(pending) > keep:
                    pending.pop(0)()

            def rope_rms(ps, cos4, sin4, out_bf, eps_ap, sqrt_scale):
                """ps: [128, 512] psum f32 (4 heads). Writes normalized bf16
                rope output to out_bf [128, 4, 128]."""
                v3 = ps.rearrange("p (h d) -> p h d", h=4)
                v4d = ps.rearrange("p (h two d) -> p h two d", h=4, two=2)
                ro = epipool.tile([P, 4, HD], f32, tag="ro", name="ro")
                cs = epipool.tile([P, 4, HD], f32, tag="cs", name="cs")
                sn = epipool.tile([P, 4, HD], f32, tag="sn", name="sn")
                cosb = bcast_sbuf(cos4[:, :, :], 2, 2)
                sinb = bcast_sbuf(sin4[:, :, :], 2, 2)
                cs4d = cs.rearrange("p h (two d) -> p h two d", two=2)
                sn4d = sn.rearrange("p h (two d) -> p h two d", two=2)
                nc.vector.tensor_tensor(cs4d, v4d, cosb, op=OP.mult)
                nc.vector.tensor_tensor(sn4d, v4d, sinb, op=OP.mult)
                nc.vector.tensor_tensor(ro[:, :, 0:64], cs[:, :, 0:64], sn[:, :, 64:128], op=OP.add)
                nc.vector.tensor_sub(ro[:, :, 64:128], cs[:, :, 64:128], sn[:, :, 0:64])
                ss = smallf.tile([P, 4], f32, tag="ss", name="ss")
                sq = epipool.tile([P, 4, HD], f32, tag="cs", name="sq")
                nc.vector.tensor_tensor(sq, ro, ro, op=OP.mult)
                nc.vector.reduce_sum(ss, sq, axis=AX.X)
                rms = smallf.tile([P, 4], f32, tag="rms", name="rms")
                nc.scalar.activation(rms, ss, AF.Sqrt, bias=eps_ap, scale=sqrt_scale)
                rinv = smallf.tile([P, 4], f32, tag="rms", name="rinv")
                nc.vector.reciprocal_approx_fast(rinv, rms)
                nc.vector.tensor_tensor(
                    out_bf, ro, bcast_sbuf(rinv[:, :], 2, HD), op=OP.mult
                )

            def pack_transpose(src_bf, dst):
                """src_bf [128, 4, 128] bf16 -> 4 PE transposes -> one copy to
                dst ([128, 4, 128] view)."""
                ptr = psY.tile([P, 512], bf16, tag="Y", name="ptrq")
                for hh in range(4):
                    nc.tensor.transpose(
                        ptr[:, hh * P:(hh + 1) * P], src_bf[:, hh, :], ident
                    )
                nc.vector.tensor_copy(dst, ptr.rearrange("p (s n) -> p s n", s=4))

            def cos_tiles(cap, sap, t0, name):
                cos4 = cspool.tile([P, 4, 64], f32, tag="cs4", name=f"c{name}")
                sin4 = cspool.tile([P, 4, 64], f32, tag="sn4", name=f"s{name}")
                nc.gpsimd.dma_start(out=cos4, in_=bcast4(cap[t0:t0 + P, :]))
                nc.gpsimd.dma_start(out=sin4, in_=bcast4(sap[t0:t0 + P, :]))
                return cos4, sin4

            # ---------------- phase 0k + 1KV: own-key shard K/V + AllGather --
            # tag-shared with xoT below: xkT is dead once K/V proj is done
            xkT = [
                bigpool.tile([P, 4, QROWS], bf16, tag=f"xT{gr}", name=f"xkT{gr}")
                for gr in range(4)
            ]
            for tt in range(NQT):
                x_transpose(xk[tt * P:(tt + 1) * P, :], xkT, tt * P, f"xk{tt}")

            wslk = load_w_slab(wk, 0, "wk")
            wslv = load_w_slab(wv, 0, "wv")
            for tt in range(NQT):
                ps = psS.tile([P, 512], f32, tag="S", name="psk")
                for kt in range(NCT):
                    nc.tensor.matmul(
                        ps,
                        xkT[kt // 4][:, kt % 4, tt * P:(tt + 1) * P],
                        wslk[:, kt, :],
                        start=(kt == 0),
                        stop=(kt == NCT - 1),
                    )
                cos4, sin4 = cos_tiles(cosk, sink, tt * P, f"k{tt}")
                khat = qhpool.tile([P, 4, HD], bf16, tag="qhat", name="khat")
                # fold attn scale into k's rms: 1/sqrt(ss + 128*eps)
                rope_rms(ps, cos4, sin4, khat, eps_k, 1.0)
                drain_pending(1)
                pending.append(
                    lambda khat=khat, tt=tt: pack_transpose(
                        khat, kOwn[:, :, tt * P:(tt + 1) * P]
                    )
                )
                psv = psS.tile([P, 512], f32, tag="S", name="psv")
                for kt in range(NCT):
                    nc.tensor.matmul(
                        psv,
                        xkT[kt // 4][:, kt % 4, tt * P:(tt + 1) * P],
                        wslv[:, kt, :],
                        start=(kt == 0),
                        stop=(kt == NCT - 1),
                    )
                nc.scalar.copy(
                    vOwn[:, tt, :, :], psv.rearrange("p (h d) -> p h d", h=4)
                )
            drain_pending()

            # bounce own K-hat/V shard to DRAM, AllGather within batch group,
            # scatter the gathered shards into the full kT / vA tiles
            nc.sync.dma_start(
                out=kv_in[:, 0:2048], in_=kOwn.rearrange("p a b -> p (a b)")
            )
            nc.sync.dma_start(
                out=kv_in[:, 2048:4096], in_=vOwn.rearrange("p a b c -> p (a b c)")
            )
            nc.gpsimd.collective_compute(
                "AllGather",
                OP.bypass,
                replica_groups=[[0, 1, 2, 3], [4, 5, 6, 7]],
                ins=[kv_in.opt()],
                outs=[kv_out.opt()],
            )
            for r in range(4):
                nc.sync.dma_start(
                    out=kT[:, :, r * QROWS:(r + 1) * QROWS],
                    in_=kv_out[r, :, 0:2048].rearrange("p (a b) -> p a b", a=NKV),
                )
                nc.sync.dma_start(
                    out=vA[:, 4 * r:4 * r + 4, :, :],
                    in_=kv_out[r, :, 2048:4096].rearrange(
                        "p (t h d) -> p t h d", t=4, h=NKV
                    ),
                )

            # ---------------- phase 0a + 1Q: own-row transposes, Q proj ----
            xoT = [
                bigpool.tile([P, 4, QROWS], bf16, tag=f"xT{gr}", name=f"xoT{gr}")
                for gr in range(4)
            ]
            for tt in range(NQT):
                x_transpose(xo[tt * P:(tt + 1) * P, :], xoT, tt * P, f"o{tt}")

            qcs = {tt: cos_tiles(coso, sino, tt * P, f"q{tt}") for tt in range(NQT)}
            for s in range(4):
                wsl = load_w_slab(wq, s * 512, f"wq{s}")
                for tt in range(NQT):
                    ps = psS.tile([P, 512], f32, tag="S", name="psq")
                    for kt in range(NCT):
                        nc.tensor.matmul(
                            ps,
                            xoT[kt // 4][:, kt % 4, tt * P:(tt + 1) * P],
                            wsl[:, kt, :],
                            start=(kt == 0),
                            stop=(kt == NCT - 1),
                        )
                    cos4, sin4 = qcs[tt]
                    qhat = qhpool.tile([P, 4, HD], bf16, tag="qhat", name="qhat")
                    rope_rms(ps, cos4, sin4, qhat, eps_q, 1.0 / HD)
                    drain_pending(1)
                    pending.append(
                        lambda qhat=qhat, s=s, tt=tt: pack_transpose(
                            qhat,
                            qT[:, 4 * s:4 * s + 4, (3 - tt) * P:(4 - tt) * P],
                        )
                    )
            drain_pending()

            # ---------------- phase 2: attention (scores-transposed) -------
            # Head-major: for each head, one variable-width matmul per
            # key-tile streams all still-valid query slots at once
            # (kt 0-3 -> N=512 ... kt 12-15 -> N=128). yt/den accumulate all
            # four slots per head in single psum tiles.
            tail_state = []  # (yt_psum, den_psum, h)

            def emit_tail():
                if not tail_state:
                    return
                yt, den, h = tail_state.pop(0)
                rinv = smallf.tile([P, QROWS], f32, tag="rq", name="rqinv")
                nc.vector.reciprocal_approx_fast(rinv, den)
                nc.vector.tensor_tensor(yT[:, h, :], yt, rinv, op=OP.mult)

            def emit_dpv(ent, yt, den, kvh):
                ppt, pkt, pn = ent
                last = pkt == NT - 1
                nc.tensor.matmul(
                    den[:, 0:pn], ones128, ppt[:, 0:pn],
                    start=(pkt == 0), stop=last, skip_group_check=True,
                )
                nc.tensor.matmul(
                    yt[:, 0:pn], vA[:, pkt, kvh, :], ppt[:, 0:pn],
                    start=(pkt == 0), stop=last, skip_group_check=True,
                )

            def load_wo_slab(s3):
                w3 = wopool.tile([P, NCT, 512], bf16, tag="wo3", name=f"wo{s3}")
                for gr in range(4):
                    nc.gpsimd.dma_start(
                        out=w3[:, 4 * gr:4 * gr + 4, :],
                        in_=wo[:, s3 * 512:s3 * 512 + 512].rearrange(
                            "(a p) n -> p a n", p=P
                        )[:, 4 * gr:4 * gr + 4, :],
                    )
                return w3

            # prefetch the first two wo slabs; their DMAs run under phase 2
            w3s = {0: load_wo_slab(0), 1: load_wo_slab(1)}

            for h in range(NH):
                kvh = h // (NH // NKV)
                yt = psY.tile([P, QROWS], f32, tag="Y", name="yt")
                den = psD.tile([P, QROWS], f32, tag="D", name="den")
                dq = []  # exp'd tiles awaiting den/PV (2-deep pipeline)
                for kt in range(NT):
                    # q-slot columns are stored high-slot-first, so the
                    # still-valid slots for key tile kt are columns [0, n)
                    n = QROWS - (kt // 4) * P
                    S = psS.tile([P, 512], f32, tag="S", name="Sb")
                    nc.tensor.matmul(
                        S[:, 0:n],
                        kT[:, kvh, kt * P:(kt + 1) * P],
                        qT[:, h, 0:n],
                        start=True,
                        stop=True,
                        skip_group_check=True,
                    )
                    if kt == 0 and tail_state:
                        emit_tail()
                    # attn scale already folded into k's rms normalization
                    pt = ptpool.tile([P, 512], bf16, tag="pt", name="pt")
                    nc.scalar.activation(pt[:, 0:n], S[:, 0:n], AF.Exp, scale=1.0)
                    # causal mask for the diagonal slot (last 128 valid cols):
                    # multiplicative 0/1 mask applied post-exp on the DVE
                    nc.vector.tensor_tensor(
                        pt[:, n - P:n], pt[:, n - P:n],
                        qmask[:, kt // 4, kt % 4, :], op=OP.mult,
                    )
                    dq.append((pt, kt, n))
                    if len(dq) > 2:
                        emit_dpv(dq.pop(0), yt, den, kvh)
                while dq:
                    emit_dpv(dq.pop(0), yt, den, kvh)
                tail_state.append((yt, den, h))
            emit_tail()

            # ---------------- phase 3: output projection ----------------
            for s3 in range(4):
                w3 = w3s.pop(s3)
                if s3 + 2 < 4:
                    w3s[s3 + 2] = load_wo_slab(s3 + 2)
                for qt in range(4):
                    ps = psS.tile([P, 512], f32, tag="S", name="ps3")
                    for ct in range(NCT):
                        nc.tensor.matmul(
                            ps,
                            yT[:, ct, (3 - qt) * P:(4 - qt) * P],
                            w3[:, ct, :],
                            start=(ct == 0),
                            stop=(ct == NCT - 1),
                        )
                    ot = outpool.tile([P, 512], f32, tag="ot", name="ot")
                    nc.vector.tensor_copy(ot, ps)
                    nc.sync.dma_start(
                        out=yo[qt * P:(qt + 1) * P, s3 * 512:(s3 + 1) * 512],
                        in_=ot,
                    )

    nc.compile()
    return nc


def _get_nc():
    if "nc" not in _CACHE:
        _CACHE["nc"] = _build()
    return _CACHE["nc"]


def _in_maps(x, cosr, sinr, wq, wk, wv, wo):
    import ml_dtypes
    bf = ml_dtypes.bfloat16
    x = x.astype(bf)
    wq, wk, wv, wo = (a.astype(bf) for a in (wq, wk, wv, wo))
    maps = []
    for core in range(8):
        b, g = core // 4, core % 4
        rows = _rows(g)
        ksl = slice(g * QROWS, (g + 1) * QROWS)
        maps.append({
            "xo": np.ascontiguousarray(x[b][rows]),
            "xk": np.ascontiguousarray(x[b][ksl]),
            "coso": np.ascontiguousarray(cosr[rows]),
            "sino": np.ascontiguousarray(sinr[rows]),
            "cosk": np.ascontiguousarray(cosr[ksl]),
            "sink": np.ascontiguousarray(sinr[ksl]),
            "wq": wq, "wk": wk, "wv": wv, "wo": wo,
            "qm": _qmask_t(g).astype(ml_dtypes.bfloat16),
        })
    return maps


def kernel(x, cos, sin, wq, wk, wv, wo):
    from concourse.bass_utils import run_bass_kernel_spmd

    x = np.ascontiguousarray(np.asarray(x, np.float32))
    cosr = np.ascontiguousarray(np.asarray(cos, np.float32).reshape(T, HD // 2))
    sinr = np.ascontiguousarray(np.asarray(sin, np.float32).reshape(T, HD // 2))
    wq = np.ascontiguousarray(np.asarray(wq, np.float32))
    wk = np.ascontiguousarray(np.asarray(wk, np.float32))
    wv = np.ascontiguousarray(np.asarray(wv, np.float32))
    wo = np.ascontiguousarray(np.asarray(wo, np.float32))

    nc = _get_nc()
    maps = _in_maps(x, cosr, sinr, wq, wk, wv, wo)
    _CACHE["in_maps"] = maps
    res = run_bass_kernel_spmd(nc, maps, list(range(8)))
    y = np.empty((B, T, C), np.float32)
    for core in range(8):
        b, g = core // 4, core % 4
        y[b][_rows(g)] = res.results[core]["yo"]
    return y
